# revision 6
# baseline (speedup 1.0000x reference)
"""AdaConv (low-rank dynamic conv) on 8 trn2 NeuronCores.

out[b,o,h,w] = sum_e para[b,e,h,w] * conv3x3(x, W_e)[b,o,h,w]
  para = conv3x3(relu(conv1x1(x, pw) + pb), cw) + cb          (16 bases)
  W_e  = basis weight e reshaped from W (64 out, 64 in, 3x3)

Sharding: pure data parallel, 8 shards = (batch b, image half hh).
Each core computes out rows [hh*64, hh*64+64) of image b from x rows
[hh*64-1, hh*64+65) (1-row halo), zero padded at image borders.

Per-core kernel (pixel-partition layout):
  - x arrives channel-major, width-padded to 130, bf16: xb (65, 66*130)
    (partition 64 = ones row -> folds pb bias into the conv1 matmul).
  - conv1: psum(16, N) = pwm(65,16).T @ xb -> relu -> pb1 (16, L) bf16.
  - im2col by shifted SBUF->SBUF DMA copies: xc chunks (128, LC) pack
    2 taps x 64 ch per chunk (5 chunks for 9 taps); pc8/pt8 likewise
    pack 8+1 taps x 16 ch of pb1.
  - per output row h (128 pixels): z psum (128pix, 1024) accumulates
    5 chunk matmuls per 512-wide half; para psum (128pix, 16) from 2
    matmuls; epilogue multiplies z e-slabs by para columns (ScalarE
    activation-scale + VectorE tensor_scalar) and tree-adds 16 slabs.
  - out written pixel-major (8192, 64) f32; host transposes.
"""

import numpy as np
import ml_dtypes

import concourse.bass as bass
import concourse.mybir as mybir
import concourse.tile as tile
from concourse import bacc
from concourse.bass_utils import run_bass_kernel_spmd

BF16 = ml_dtypes.bfloat16

B, C, H, WD = 4, 64, 128, 128
E = 16            # bases
NCORES = 8
HALF = H // 2     # 64 output rows per core
RH = HALF + 2     # 66 stored x rows (1 halo each side)
WP = WD + 2        # 130 padded width
L = RH * WP       # 8580 columns of the padded per-core image
LC = L - 2 * (WP + 1)  # 8318: im2col span, col i <-> center index i+131
NT = HALF         # 64 row-tiles per core, 128 pixels each
OFFS = [dh * WP + dw for dh in (-1, 0, 1) for dw in (-1, 0, 1)]  # 9 tap offsets
NBAND = 4
BAND_ROWS = 16
BAND = BAND_ROWS * WP  # 2080 im2col cols per band


def _prep_weights(Wt, pw, pb, cw, cb):
    """Host-side relayout of all weights (small, replicated to all cores)."""
    T = np.asarray(Wt, np.float32).reshape(C, C, 9, E)   # [o, c, tap, e]
    A = T.transpose(2, 1, 3, 0).reshape(9, C, E * C)     # [tap, c, (e*64+o)]
    wm = [np.concatenate([A[2 * k], A[2 * k + 1]], axis=0) for k in range(4)]
    wm.append(A[8])                                      # (64, 1024)

    pwm = np.concatenate(
        [np.asarray(pw, np.float32).reshape(E, C).T,
         np.asarray(pb, np.float32).reshape(1, E)], axis=0)   # (65, 16)

    A2 = np.asarray(cw, np.float32).transpose(2, 3, 1, 0).reshape(9, E, E)
    cw8 = A2[:8].reshape(8 * E, E)                            # (128, 16)
    # row 0 = cb (pairs with the ones row at partition 0 of pt8)
    cw1 = np.concatenate(
        [np.asarray(cb, np.float32).reshape(1, E), A2[8]], axis=0)  # (17, 16)

    return ([m.astype(BF16) for m in wm], pwm.astype(BF16),
            cw8.astype(BF16), cw1.astype(BF16))


def _shard_x(x):
    """(B,C,H,W) f32 -> 8 shards (65, L) bf16, channel-major padded, +ones row."""
    xp = np.zeros((B, C, H + 2, WP), np.float32)
    xp[:, :, 1:H + 1, 1:WD + 1] = x
    shards = []
    for b in range(B):
        for hh in range(2):
            rows = xp[b, :, hh * HALF: hh * HALF + RH, :].reshape(C, L)
            sh = np.concatenate([rows, np.ones((1, L), np.float32)], axis=0)
            shards.append(sh.astype(BF16))
    return shards


def build_bass():
    f32 = mybir.dt.float32
    bf16 = mybir.dt.bfloat16
    Relu = mybir.ActivationFunctionType.Relu
    Copy = mybir.ActivationFunctionType.Copy

    nc = bacc.Bacc("TRN2", target_bir_lowering=False, debug=False,
                   num_devices=NCORES)

    x_d = nc.declare_dram_parameter("x", [C + 1, L], bf16, isOutput=False)
    wm_d = [nc.declare_dram_parameter(f"wm{k}", [128 if k < 4 else 64, E * C],
                                      bf16, isOutput=False) for k in range(5)]
    pwm_d = nc.declare_dram_parameter("pwm", [C + 1, E], bf16, isOutput=False)
    cw8_d = nc.declare_dram_parameter("cw8", [8 * E, E], bf16, isOutput=False)
    cw1_d = nc.declare_dram_parameter("cw1", [E + 1, E], bf16, isOutput=False)
    out_d = nc.declare_dram_parameter("out", [HALF * WD, C], f32, isOutput=True)

    with tile.TileContext(nc) as tc:
        with (
            tc.tile_pool(name="const", bufs=1) as constp,
            tc.tile_pool(name="big", bufs=1) as bigp,
            tc.tile_pool(name="work", bufs=3) as workp,
            tc.tile_pool(name="ps_z", bufs=2, space="PSUM") as zpool,
            tc.tile_pool(name="ps_p", bufs=2, space="PSUM") as ppool,
            tc.tile_pool(name="ps_1", bufs=2, space="PSUM") as p1pool,
        ):
            # --- weights to SBUF ---
            wm = []
            for k in range(5):
                t = constp.tile([128 if k < 4 else 64, E * C], bf16,
                                name=f"wm{k}s", tag=f"wm{k}")
                nc.sync.dma_start(t[:], wm_d[k].ap())
                wm.append(t)
            pwm = constp.tile([C + 1, E], bf16, tag="pwm")
            nc.sync.dma_start(pwm[:], pwm_d.ap())
            cw8 = constp.tile([8 * E, E], bf16, tag="cw8")
            nc.sync.dma_start(cw8[:], cw8_d.ap())
            cw1 = constp.tile([E + 1, E], bf16, tag="cw1")
            nc.sync.dma_start(cw1[:], cw1_d.ap())

            # --- x to SBUF (channel-major, padded, with ones row) ---
            xb = bigp.tile([C + 1, L], bf16, tag="xb")
            nc.sync.dma_start(xb[:], x_d.ap())

            # --- conv1 + relu -> pb1 (16, L) bf16 ---
            pb1 = bigp.tile([E, L], bf16, tag="pb1")
            n1 = (L + 511) // 512
            for i in range(n1):
                c0 = i * 512
                n = min(512, L - c0)
                p1 = p1pool.tile([E, 512], f32, tag="p1")
                nc.tensor.matmul(p1[:, :n], pwm[:], xb[:, c0:c0 + n],
                                 start=True, stop=True)
                nc.scalar.activation(pb1[:, c0:c0 + n], p1[:, :n], Relu)

            # --- im2col buffers, built banded so tiles can start early ---
            xc = [bigp.tile([128 if k < 4 else 64, LC], bf16,
                            name=f"xc{k}", tag=f"xc{k}") for k in range(5)]
            pc8 = bigp.tile([8 * E, LC], bf16, tag="pc8")
            pt8 = bigp.tile([E + 1, LC], bf16, tag="pt8")
            for bd in range(NBAND):
                c0 = bd * BAND
                n = min(BAND, LC - c0)
                for t in range(9):
                    k, j = divmod(t, 2)
                    src = xb[0:C, c0 + 131 + OFFS[t]: c0 + 131 + OFFS[t] + n]
                    nc.sync.dma_start(xc[k][64 * j:64 * j + 64, c0:c0 + n], src)
                for t in range(8):
                    nc.sync.dma_start(
                        pc8[E * t:E * t + E, c0:c0 + n],
                        pb1[:, c0 + 131 + OFFS[t]: c0 + 131 + OFFS[t] + n])
                nc.sync.dma_start(
                    pt8[1:E + 1, c0:c0 + n],
                    pb1[:, c0 + 131 + OFFS[8]: c0 + 131 + OFFS[8] + n])
                nc.gpsimd.memset(pt8[0:1, c0:c0 + n], 1.0)

            # --- main per-row-tile loop ---
            for h in range(NT):
                c0 = h * WP  # im2col col of first active pixel of row h

                # predictor conv2 -> para (128 pix, 16)
                pp = ppool.tile([128, E], f32, tag="pp")
                nc.tensor.matmul(pp[:], pc8[:, c0:c0 + 128], cw8[:],
                                 start=True, stop=False)
                nc.tensor.matmul(pp[:], pt8[:, c0:c0 + 128], cw1[:],
                                 start=False, stop=True)
                pa = workp.tile([128, E], f32, tag="pa")
                nc.vector.tensor_copy(pa[:], pp[:])

                # z (128 pix, 1024 eo), 5-chunk accumulation per 512-half
                zp = zpool.tile([128, E * C], f32, tag="zp")
                for half in range(2):
                    for k in range(5):
                        nc.tensor.matmul(
                            zp[:, 512 * half:512 * half + 512],
                            xc[k][:, c0:c0 + 128],
                            wm[k][:, 512 * half:512 * half + 512],
                            start=(k == 0), stop=(k == 4))

                # epilogue: m[:, e*64:+64] = zp_slab * para_e
                m = workp.tile([128, E * C], bf16, tag="m")
                for e in range(E):
                    sl = slice(C * e, C * e + C)
                    if e < 8:   # bank 0 of zp -> ScalarE
                        nc.scalar.activation(m[:, sl], zp[:, sl], Copy,
                                             scale=pa[:, e:e + 1])
                    else:       # bank 1 of zp -> VectorE
                        nc.vector.tensor_scalar_mul(m[:, sl], zp[:, sl],
                                                    pa[:, e:e + 1])
                s1 = workp.tile([128, 512], bf16, tag="s1")
                nc.vector.tensor_add(s1[:], m[:, 0:512], m[:, 512:1024])
                s2 = workp.tile([128, 256], bf16, tag="s2")
                nc.vector.tensor_add(s2[:], s1[:, 0:256], s1[:, 256:512])
                s3 = workp.tile([128, 128], bf16, tag="s3")
                nc.vector.tensor_add(s3[:], s2[:, 0:128], s2[:, 128:256])
                ot = workp.tile([128, C], f32, tag="ot")
                nc.vector.tensor_add(ot[:], s3[:, 0:64], s3[:, 64:128])

                nc.sync.dma_start(out_d.ap()[128 * h:128 * h + 128, :], ot[:])

    nc.compile()
    return nc


_CACHE = {}


def _get_nc():
    if "nc" not in _CACHE:
        _CACHE["nc"] = build_bass()
    return _CACHE["nc"]


def kernel(x, W, pw, pb, cw, cb):
    x = np.asarray(x, np.float32)
    wm, pwm, cw8, cw1 = _prep_weights(W, pw, pb, cw, cb)
    shards = _shard_x(x)

    base = {f"wm{k}": wm[k] for k in range(5)}
    base.update(pwm=pwm, cw8=cw8, cw1=cw1)
    in_maps = [dict(base, x=shards[i]) for i in range(NCORES)]

    nc = _get_nc()
    res = run_bass_kernel_spmd(nc, in_maps, core_ids=list(range(NCORES)))

    out = np.empty((B, C, H, WD), np.float32)
    for i in range(NCORES):
        b, hh = divmod(i, 2)
        sh = res.results[i]["out"]  # (8192, 64) pixel-major
        out[b, :, hh * HALF:(hh + 1) * HALF, :] = (
            sh.reshape(HALF, WD, C).transpose(2, 0, 1))
    return out
